# revision 1
# baseline (speedup 1.0000x reference)
"""DLRM DCN kernel for 8 TRN2 NeuronCores — batch-sharded (data parallel).

Per core c (bags [c*512, (c+1)*512) of every table):
  - embedding rows gathered with dma_gather (bf16 tables; each table split
    into 4 row-groups of 25000 rows so indices fit int16; padding slots
    point at a zero row),
  - bags are assigned to 8 subgroups of 64 by a balancing permutation
    (inverted on the host via dense-input / output reordering),
  - pooling on TensorE: psum_T[d, sg*64:+64] (+)= chunk[128rows, d].T @ S
    with S a shipped one-hot (fp8 in DRAM, cast to bf16 during DMA),
  - bottom MLP / 3 low-rank DCN layers / top MLP in bf16, fp32 PSUM.
"""
import numpy as np
import ml_dtypes

import concourse.bass as bass
import concourse.tile as tile
from concourse import bacc, mybir
from concourse._compat import with_exitstack

# ---- problem constants ----
B, T, D = 4096, 26, 128
ROWS = 100000
NBLK = 27                  # 1 bottom block + 26 tables
F_IN = NBLK * D            # 3456
NCORES = 8
BC = B // NCORES           # 512 bags per core
SGB = 64                   # bags per subgroup (psum band width)
SG = BC // SGB             # 8
RG = 4                     # row groups per table (int16 index limit)
RGR = ROWS // RG           # 25000 real rows per group
RGB = RGR + 24             # block size; rows >= RGR are zeros (pad target)
ZPAD = RGR                 # pad index (zero row)
LOW_RANK = 512
P = 128

bf16_np = ml_dtypes.bfloat16
fp8_np = ml_dtypes.float8_e4m3
f32 = mybir.dt.float32
bf16 = mybir.dt.bfloat16
i16 = mybir.dt.int16
fp8 = mybir.dt.float8e4


# ======================================================================
# host-side preprocessing
# ======================================================================

def _seg_ids(offsets_t, nt):
    seg = np.searchsorted(offsets_t, np.arange(nt), side="right") - 1
    return np.clip(seg, 0, B - 1)


def _balance(lens_core):
    """lens_core [T, BC] bag lengths -> assign [BC] subgroup ids (64 each),
    greedy LPT minimizing max-over-tables subgroup load."""
    tot = lens_core.sum(axis=0)
    order = np.argsort(-tot, kind="stable")
    loads = np.zeros((SG, T))
    cap = np.zeros(SG, np.int64)
    assign = np.zeros(BC, np.int64)
    for b in order:
        best, bestscore = -1, None
        for g in range(SG):
            if cap[g] >= SGB:
                continue
            score = (loads[g] + lens_core[:, b]).max()
            if bestscore is None or score < bestscore:
                best, bestscore = g, score
        assign[b] = best
        cap[best] += 1
        loads[best] += lens_core[:, b]
    return assign


def prep(inputs):
    indices = np.asarray(inputs["indices"], np.int64)   # [T, NT]
    offsets = np.asarray(inputs["offsets"], np.int64)   # [T, B]
    NT = indices.shape[1]

    segs, lens = [], np.zeros((T, B), np.int64)
    for t in range(T):
        s = _seg_ids(offsets[t], NT)
        segs.append(s)
        lens[t] = np.bincount(s, minlength=B)

    # --- balanced bag -> slot permutation per core ---
    # slot[b] = subgroup*64 + rank; perm[s] = original local bag at slot s
    slot_of = np.zeros((NCORES, BC), np.int64)
    perm = np.zeros((NCORES, BC), np.int64)
    for c in range(NCORES):
        assign = _balance(lens[:, c * BC:(c + 1) * BC])
        slot = np.zeros(BC, np.int64)
        nxt = np.zeros(SG, np.int64)
        for b in range(BC):
            g = assign[b]
            slot[b] = g * SGB + nxt[g]
            nxt[g] += 1
        slot_of[c] = slot
        perm[c, slot] = np.arange(BC)

    # --- element lists per (core, table, sg, rg) ---
    counts = np.zeros((NCORES, T, SG, RG), np.int64)
    lists = {}
    for t in range(T):
        seg = segs[t]
        rows = indices[t]
        rg_all = rows // RGR
        lrow_all = rows - rg_all * RGR
        for c in range(NCORES):
            lo = np.searchsorted(seg, c * BC, side="left")
            hi = np.searchsorted(seg, (c + 1) * BC, side="left")
            sl = slot_of[c][seg[lo:hi] - c * BC]
            sg = sl // SGB
            b64 = sl % SGB
            rg = rg_all[lo:hi]
            lrow = lrow_all[lo:hi]
            for g in range(SG):
                for r in range(RG):
                    m = (sg == g) & (rg == r)
                    lists[(c, t, g, r)] = (lrow[m], b64[m])
                    counts[c, t, g, r] = m.sum()

    K = np.maximum((counts.max(axis=0) + P - 1) // P, 1)  # [T, SG, RG]
    C_trg = K.sum(axis=1)          # [T, RG]
    TOTC = int(C_trg.sum())

    gidx = np.zeros((NCORES, P, TOTC * 8), np.int16)
    smat = np.zeros((NCORES, P, TOTC, SGB), np.float32)
    col_meta = []                  # per (t, rg): (col0, ncols, sg_of_col)
    col0 = 0
    for t in range(T):
        for r in range(RG):
            ncols = int(C_trg[t, r])
            sg_of_col = []
            for g in range(SG):
                sg_of_col += [g] * int(K[t, g, r])
            col_meta.append((t, r, col0, ncols, sg_of_col))
            for c in range(NCORES):
                lr_parts, b_parts = [], []
                for g in range(SG):
                    lr, b64 = lists[(c, t, g, r)]
                    cap = int(K[t, g, r]) * P
                    padn = cap - len(lr)
                    lr_parts.append(np.concatenate(
                        [lr, np.full(padn, ZPAD, np.int64)]))
                    b_parts.append(np.concatenate(
                        [b64, np.full(padn, -1, np.int64)]))
                lr_all = np.concatenate(lr_parts)
                b_all = np.concatenate(b_parts)
                n = ncols * P
                w = lr_all.reshape(n // 16, 16).T.astype(np.int16)
                for gg in range(8):
                    gidx[c, gg * 16:(gg + 1) * 16,
                         col0 * 8:col0 * 8 + n // 16] = w
                valid = b_all >= 0
                kk = np.arange(n)[valid]
                smat[c, kk % P, col0 + kk // P, b_all[valid]] = 1.0
            col0 += ncols
    assert col0 == TOTC

    dense = np.asarray(inputs["dense_input"], np.float32)

    emb = np.asarray(inputs["emb_tables"], np.float32)   # [T, ROWS, D]
    emb_dev = np.zeros((T * RG * RGB, D), bf16_np)
    ev = emb_dev.reshape(T, RG, RGB, D)
    ev[:, :, :RGR, :] = emb.reshape(T, RG, RGR, D).astype(bf16_np)

    def chunkT(W, kdim, mdim):
        # W [mdim, kdim] -> [128, kdim//128, mdim]
        kc = kdim // P
        return np.ascontiguousarray(
            W.reshape(mdim, kc, P).transpose(2, 1, 0)).astype(bf16_np)

    wts = {}
    wts["botW0T"] = np.asarray(inputs["bot_W0"], np.float32).T.astype(bf16_np)
    wts["botW1T"] = chunkT(np.asarray(inputs["bot_W1"], np.float32), 512, 256)
    wts["botW2T"] = chunkT(np.asarray(inputs["bot_W2"], np.float32), 256, 128)
    dcn_V = np.asarray(inputs["dcn_V"], np.float32)
    dcn_W = np.asarray(inputs["dcn_W"], np.float32)
    wts["VT"] = np.stack([chunkT(dcn_V[l], F_IN, LOW_RANK) for l in range(3)])
    wts["WT"] = np.stack([chunkT(dcn_W[l], LOW_RANK, F_IN) for l in range(3)])
    top_W0 = np.asarray(inputs["top_W0"], np.float32)
    wts["topW0T"] = np.stack([chunkT(top_W0[h * 512:(h + 1) * 512], F_IN, 512)
                              for h in range(2)])
    top_W1 = np.asarray(inputs["top_W1"], np.float32)
    wts["topW1T"] = np.stack([chunkT(top_W1[h * 512:(h + 1) * 512], 1024, 512)
                              for h in range(2)])        # [2, 128, 8, 512]
    wts["topW2T"] = chunkT(np.asarray(inputs["top_W2"], np.float32), 1024, 512)
    wts["topW3T"] = chunkT(np.asarray(inputs["top_W3"], np.float32), 512, 1)

    botB = np.zeros((P, 7), np.float32)
    botB[:, 0:4] = np.asarray(inputs["bot_b0"], np.float32).reshape(4, P).T
    botB[:, 4:6] = np.asarray(inputs["bot_b1"], np.float32).reshape(2, P).T
    botB[:, 6] = np.asarray(inputs["bot_b2"], np.float32)
    dcnB = np.ascontiguousarray(
        np.asarray(inputs["dcn_b"], np.float32).reshape(3, NBLK, P)
        .transpose(2, 0, 1))
    topB = np.zeros((P, 20), np.float32)
    topB[:, 0:8] = np.asarray(inputs["top_b0"], np.float32).reshape(8, P).T
    topB[:, 8:16] = np.asarray(inputs["top_b1"], np.float32).reshape(8, P).T
    topB[:, 16:20] = np.asarray(inputs["top_b2"], np.float32).reshape(4, P).T
    topb3 = np.asarray(inputs["top_b3"], np.float32).reshape(1, 1)

    shared = dict(emb=emb_dev, botB=botB, dcnB=dcnB, topB=topB, topb3=topb3,
                  **wts)
    per_core = []
    for c in range(NCORES):
        dslice = dense[c * BC + perm[c]]          # permuted batch rows
        per_core.append(dict(
            gidx=gidx[c],
            smat=smat[c].astype(fp8_np),
            denseT=np.ascontiguousarray(dslice.T).astype(bf16_np),
        ))
    return shared, per_core, col_meta, TOTC, perm


# ======================================================================
# device kernel
# ======================================================================

@with_exitstack
def build_kernel(ctx, tc, aps, col_meta):
    nc = tc.nc
    RELU = mybir.ActivationFunctionType.Relu
    COPY = mybir.ActivationFunctionType.Copy

    emb_d, gidx_d, smat_d = aps["emb"], aps["gidx"], aps["smat"]

    const = ctx.enter_context(tc.tile_pool(name="const", bufs=1))
    gpool = ctx.enter_context(tc.tile_pool(name="gath", bufs=3))
    spool = ctx.enter_context(tc.tile_pool(name="smat", bufs=3))
    ipool = ctx.enter_context(tc.tile_pool(name="gidx", bufs=3))
    xpool = ctx.enter_context(tc.tile_pool(name="xbuf", bufs=1))
    wpool = ctx.enter_context(tc.tile_pool(name="wstr", bufs=3))
    zpool = ctx.enter_context(tc.tile_pool(name="zbuf", bufs=1))
    vpool = ctx.enter_context(tc.tile_pool(name="vbuf", bufs=1))
    tpool = ctx.enter_context(tc.tile_pool(name="temps", bufs=3))
    pp_pool = ctx.enter_context(tc.tile_pool(name="ppool", bufs=2, space="PSUM"))
    pm_pool = ctx.enter_context(tc.tile_pool(name="pmlp", bufs=2, space="PSUM"))
    pv_pool = ctx.enter_context(tc.tile_pool(name="pv", bufs=4, space="PSUM"))

    combined = xpool.tile([P, NBLK, BC], bf16)
    x_a = xpool.tile([P, NBLK, BC], bf16)
    x_b = xpool.tile([P, NBLK, BC], bf16)

    botB_t = const.tile([P, 7], f32)
    nc.sync.dma_start(botB_t[:], aps["botB"][:])
    dcnB_t = const.tile([P, 3, NBLK], f32)
    nc.sync.dma_start(dcnB_t[:], aps["dcnB"][:])
    topB_t = const.tile([P, 20], f32)
    nc.sync.dma_start(topB_t[:], aps["topB"][:])
    topb3_t = const.tile([1, 1], f32)
    nc.sync.dma_start(topb3_t[:], aps["topb3"][:])

    # ---------------- bottom MLP -> combined block 0 ----------------
    denseT_t = const.tile([13, BC], bf16)
    nc.sync.dma_start(denseT_t[:], aps["denseT"][:])
    botW0T_t = const.tile([13, 512], bf16)
    nc.sync.dma_start(botW0T_t[:], aps["botW0T"][:])
    botW1T_t = const.tile([P, 4, 256], bf16)
    nc.sync.dma_start(botW1T_t[:], aps["botW1T"][:])
    botW2T_t = const.tile([P, 2, 128], bf16)
    nc.sync.dma_start(botW2T_t[:], aps["botW2T"][:])

    x1 = zpool.tile([P, 4, BC], bf16, tag="x1")
    for mt in range(4):
        ps = pm_pool.tile([P, BC], f32, tag="mlp")
        nc.tensor.matmul(ps[:], lhsT=botW0T_t[:, mt * P:(mt + 1) * P],
                         rhs=denseT_t[:], start=True, stop=True)
        nc.scalar.activation(x1[:, mt, :], ps[:], RELU,
                             bias=botB_t[:, mt:mt + 1])
    x2 = zpool.tile([P, 2, BC], bf16, tag="x2")
    for mt in range(2):
        ps = pm_pool.tile([P, BC], f32, tag="mlp")
        for kc in range(4):
            nc.tensor.matmul(ps[:], lhsT=botW1T_t[:, kc, mt * P:(mt + 1) * P],
                             rhs=x1[:, kc, :], start=(kc == 0), stop=(kc == 3))
        nc.scalar.activation(x2[:, mt, :], ps[:], RELU,
                             bias=botB_t[:, 4 + mt:5 + mt])
    ps = pm_pool.tile([P, BC], f32, tag="mlp")
    for kc in range(2):
        nc.tensor.matmul(ps[:], lhsT=botW2T_t[:, kc, :],
                         rhs=x2[:, kc, :], start=(kc == 0), stop=(kc == 1))
    nc.scalar.activation(combined[:, 0, :], ps[:], RELU,
                         bias=botB_t[:, 6:7])

    # ---------------- embedding gather + pooling ----------------
    by_table = {}
    for (t, r, col0, ncols, sg_of_col) in col_meta:
        by_table.setdefault(t, []).append((r, col0, ncols, sg_of_col))

    for t in range(T):
        psum_t = pp_pool.tile([P, BC], f32, tag="pool")
        first = True
        for (r, col0, ncols, sg_of_col) in by_table[t]:
            nidx = ncols * P
            it = ipool.tile([P, nidx // 16], i16, tag="gidx")
            nc.sync.dma_start(it[:], gidx_d[:, col0 * 8:col0 * 8 + nidx // 16])
            st = spool.tile([P, ncols, SGB], bf16, tag="smat")
            nc.gpsimd.dma_start(st[:], smat_d[:, col0:col0 + ncols, :])
            gt = gpool.tile([P, ncols, D], bf16, tag="gath")
            src = emb_d[(t * RG + r) * RGB:(t * RG + r + 1) * RGB, :]
            nc.gpsimd.dma_gather(
                out_ap=gt[:], in_ap=src, idxs_ap=it[:],
                num_idxs=nidx, num_idxs_reg=nidx, elem_size=D,
                single_packet=False)
            for j in range(ncols):
                g = sg_of_col[j]
                nc.tensor.matmul(
                    psum_t[:, g * SGB:(g + 1) * SGB],
                    lhsT=gt[:, j, :], rhs=st[:, j, :],
                    start=first, stop=(j == ncols - 1 and r == RG - 1),
                    skip_group_check=True)
                first = False
        nc.scalar.activation(combined[:, t + 1, :], psum_t[:], COPY)

    # ---------------- DCN cross layers ----------------
    # all streamed weights go through one 9-KB tag: [128, 9, 512] bf16
    x_cur = combined
    bufs = [x_a, x_b]
    for l in range(3):
        # v^T[rt] accumulated across 3 streamed VT chunks of 9 blocks
        psv = [pv_pool.tile([P, BC], f32, tag="v", name=f"psv{l}_{i}")
               for i in range(4)]
        for ci in range(3):
            w = wpool.tile([P, 9, 512], bf16, tag="w9")
            nc.sync.dma_start(w[:], aps["VT"][l, :, ci * 9:(ci + 1) * 9, :])
            for rt in range(4):
                for bi in range(9):
                    blk = ci * 9 + bi
                    nc.tensor.matmul(
                        psv[rt][:], lhsT=w[:, bi, rt * P:(rt + 1) * P],
                        rhs=x_cur[:, blk, :],
                        start=(blk == 0), stop=(blk == NBLK - 1),
                        skip_group_check=True)
        vT = vpool.tile([P, 4, BC], bf16, tag="vT")
        for rt in range(4):
            nc.scalar.activation(vT[:, rt, :], psv[rt][:], COPY)
        # w^T [F, b] blocks + elementwise update; WT streamed in 3 chunks
        # of 9 F-blocks: aps["WT"][l] is [128, 4, 27*128]
        x_next = bufs[l % 2]
        for ci in range(3):
            w = wpool.tile([P, 4, 9 * P], bf16, tag="w9")
            nc.sync.dma_start(w[:], aps["WT"][l, :, :, ci * 9 * P:(ci + 1) * 9 * P])
            for bi in range(9):
                blk = ci * 9 + bi
                ps = pm_pool.tile([P, BC], f32, tag="mlp")
                for kc in range(4):
                    nc.tensor.matmul(
                        ps[:], lhsT=w[:, kc, bi * P:(bi + 1) * P],
                        rhs=vT[:, kc, :], start=(kc == 0), stop=(kc == 3))
                wb = tpool.tile([P, BC], bf16, tag="wb")
                nc.vector.tensor_scalar_add(wb[:], ps[:],
                                            dcnB_t[:, l, blk:blk + 1])
                t2 = tpool.tile([P, BC], bf16, tag="t2")
                nc.vector.tensor_mul(t2[:], combined[:, blk, :], wb[:])
                nc.vector.tensor_add(x_next[:, blk, :], t2[:], x_cur[:, blk, :])
        x_cur = x_next

    # ---------------- top MLP ----------------
    z1 = zpool.tile([P, 8, BC], bf16, tag="z1")
    for h in range(2):
        psz1 = [pv_pool.tile([P, BC], f32, tag="v", name=f"psz1_{h}_{i}")
                for i in range(4)]
        for ci in range(3):
            w = wpool.tile([P, 9, 512], bf16, tag="w9")
            nc.sync.dma_start(w[:], aps["topW0T"][h, :, ci * 9:(ci + 1) * 9, :])
            for mt in range(4):
                for bi in range(9):
                    blk = ci * 9 + bi
                    nc.tensor.matmul(
                        psz1[mt][:], lhsT=w[:, bi, mt * P:(mt + 1) * P],
                        rhs=x_cur[:, blk, :],
                        start=(blk == 0), stop=(blk == NBLK - 1),
                        skip_group_check=True)
        for mt in range(4):
            m = h * 4 + mt
            nc.scalar.activation(z1[:, m, :], psz1[mt][:], RELU,
                                 bias=topB_t[:, m:m + 1])
    z2 = zpool.tile([P, 8, BC], bf16, tag="z2")
    for h in range(2):
        w1 = wpool.tile([P, 8, 512], bf16, tag="w9")
        nc.sync.dma_start(w1[:], aps["topW1T"][h][:])
        for mt in range(4):
            ps = pm_pool.tile([P, BC], f32, tag="mlp")
            for kc in range(8):
                nc.tensor.matmul(ps[:], lhsT=w1[:, kc, mt * P:(mt + 1) * P],
                                 rhs=z1[:, kc, :], start=(kc == 0),
                                 stop=(kc == 7))
            m = h * 4 + mt
            nc.scalar.activation(z2[:, m, :], ps[:], RELU,
                                 bias=topB_t[:, 8 + m:9 + m])
    z3 = zpool.tile([P, 4, BC], bf16, tag="z3")
    w2 = wpool.tile([P, 8, 512], bf16, tag="w9")
    nc.sync.dma_start(w2[:], aps["topW2T"][:])
    for mt in range(4):
        ps = pm_pool.tile([P, BC], f32, tag="mlp")
        for kc in range(8):
            nc.tensor.matmul(ps[:], lhsT=w2[:, kc, mt * P:(mt + 1) * P],
                             rhs=z2[:, kc, :], start=(kc == 0), stop=(kc == 7))
        nc.scalar.activation(z3[:, mt, :], ps[:], RELU,
                             bias=topB_t[:, 16 + mt:17 + mt])
    w3 = const.tile([P, 4, 1], bf16)
    nc.sync.dma_start(w3[:], aps["topW3T"][:])
    psz = pm_pool.tile([1, BC], f32, tag="mlp")
    for kc in range(4):
        nc.tensor.matmul(psz[:], lhsT=w3[:, kc, :], rhs=z3[:, kc, :],
                         start=(kc == 0), stop=(kc == 3))
    out_t = const.tile([1, BC], f32)
    nc.vector.tensor_scalar_add(out_t[:], psz[:], topb3_t[:1, :1])
    nc.sync.dma_start(aps["z"][:], out_t[:])


# ======================================================================
# driver
# ======================================================================

def make_nc(col_meta, TOTC):
    nc = bacc.Bacc("TRN2", target_bir_lowering=False, debug=False)
    aps = {}
    aps["emb"] = nc.dram_tensor("emb", [T * RG * RGB, D], bf16,
                                kind="ExternalInput").ap()
    aps["gidx"] = nc.dram_tensor("gidx", [P, TOTC * 8], i16,
                                 kind="ExternalInput").ap()
    aps["smat"] = nc.dram_tensor("smat", [P, TOTC, SGB], fp8,
                                 kind="ExternalInput").ap()
    aps["denseT"] = nc.dram_tensor("denseT", [13, BC], bf16,
                                   kind="ExternalInput").ap()
    for name, shape, dt in [
        ("botW0T", [13, 512], bf16), ("botW1T", [P, 4, 256], bf16),
        ("botW2T", [P, 2, 128], bf16),
        ("VT", [3, P, NBLK, 512], bf16), ("WT", [3, P, 4, F_IN], bf16),
        ("topW0T", [2, P, NBLK, 512], bf16), ("topW1T", [2, P, 8, 512], bf16),
        ("topW2T", [P, 8, 512], bf16), ("topW3T", [P, 4, 1], bf16),
        ("botB", [P, 7], f32), ("dcnB", [P, 3, NBLK], f32),
        ("topB", [P, 20], f32), ("topb3", [1, 1], f32),
    ]:
        aps[name] = nc.dram_tensor(name, shape, dt, kind="ExternalInput").ap()
    aps["z"] = nc.dram_tensor("z", [1, BC], f32, kind="ExternalOutput").ap()

    with tile.TileContext(nc) as tc:
        build_kernel(tc, aps, col_meta)
    nc.compile()
    return nc


def kernel(**inputs):
    from concourse.bass_utils import run_bass_kernel_spmd
    shared, per_core, col_meta, TOTC, perm = prep(inputs)
    nc = make_nc(col_meta, TOTC)
    in_maps = [{**shared, **pc} for pc in per_core]
    res = run_bass_kernel_spmd(nc, in_maps, list(range(NCORES)), trace=False)
    z = np.empty((B,), np.float32)
    for c in range(NCORES):
        zc = res.results[c]["z"][0]          # slot order
        z[c * BC + perm[c]] = zc
    return z.reshape(B, 1)

